# revision 6
# baseline (speedup 1.0000x reference)
"""Luong attention pooling kernel v2 for Trainium2 (Bass/Tile), 8 NeuronCores.

Problem (full shapes, fp32):
    decoder_state:   [32, 512]
    encoder_hiddens: [32, 8192, 512]
    scores  = einsum('bd,bsd->bs'); attn = softmax(scores, axis=1)
    context = einsum('bs,bsd->bd')

Sharding: data-parallel over batch; each of the 8 cores handles 4 batches
independently (no collectives).

Per-core pipeline (HBM-read-bound: ~200 us floor for the 64 MiB stream):
  - stream 64 f32 tiles [128s x 512d] per batch on the SP HWDGE queue
  - per tile, ONE fused DVE op: G16[t] = (H[t] * dec_b) cast to fp16, with
    accum_out giving the 128 scores. G16 is kept in a deep fp16 ring; the
    f32 tile is freed immediately (only DVE reads it).
  - per batch (no segments): rowmax -> PE transpose vs -I -> min = -m ->
    PE broadcast -> ACT exp(scores - m) -> attn16 [128,64] fp16 with fused
    row-sum r_p; L via tiny PE matmul; 64 accumulating PE matmuls
    ctx' = sum_t attn16[:,t]^T @ G16[t]  (= context * dec elementwise)
  - final: ctx = ctx' * (1/L) / dec  (one fused DVE op; division undoes the
    dec factor folded into G; dec ~ N(0,1) so the error stays relative)
  - the final scale/store for batch b is emitted AFTER batch b+1's score
    ops so DVE's in-order stream never stalls on the PE context matmuls.

Engine budgets per iteration (4 batches): DMA ~200 us (floor), DVE ~150 us,
PE ~100 us, ACT ~5 us. Target: the DMA floor.
"""

import numpy as np

import bass_rust
import concourse.bass as bass
import concourse.tile as tile
from concourse import mybir
from concourse.bass_utils import run_bass_kernel_spmd

N_CORES = 8
B_TOTAL = 32
S = 8192
D = 512
B = B_TOTAL // N_CORES
P = 128
T = S // P  # 64 s-tiles per batch

F32 = mybir.dt.float32
FP16 = mybir.dt.float16

F32_BUFS = 40  # staging ring of f32 tiles (2 KiB/partition each)
G16_BUFS = 96  # fp16 G ring (1 KiB/partition each) — 1.5 batches deep


def _legalize_waits(nc: bass.Bass, max_inline: int = 1) -> int:
    """The walrus build here accepts at most one sync wait per regular
    instruction; hoist Tile's multi-waits into standalone EventSemaphore
    instructions immediately before the instruction."""
    n = 0
    for f in nc.m.functions:
        for b in f.blocks:
            changed = False
            new = []
            for i in b.instructions:
                si = i.sync_info
                waits = list(si.on_wait) if si is not None else []
                if len(waits) > max_inline:
                    for k, w in enumerate(waits[max_inline:]):
                        es = mybir.InstEventSemaphore(
                            name=f"{i.name}-hw{k}", engine=i.engine, ins=[], outs=[]
                        )
                        es.sync_info = bass_rust.SyncInfo(on_wait=[w], on_update=[])
                        new.append(es)
                        n += 1
                    i.sync_info = bass_rust.SyncInfo(
                        on_wait=waits[:max_inline], on_update=list(si.on_update)
                    )
                    changed = True
                new.append(i)
            if changed:
                b.instructions = new
    return n


def build_nc(repeat: int = 1, mode: str = "full", loop: int | None = None) -> bass.Bass:
    """repeat: unrolled iterations; loop: optional For_i hardware loop around
    them (benchmarking only). mode: "full" | "dma" | "dma+dve"."""
    nc = bass.Bass()
    dec = nc.declare_dram_parameter("decoder_state", [B, D], F32, isOutput=False)
    enc = nc.declare_dram_parameter("encoder_hiddens", [B, S, D], F32, isOutput=False)
    out = nc.declare_dram_parameter("context", [B, D], F32, isOutput=True)

    with tile.TileContext(nc) as tc:
        with (
            tc.tile_pool(name="hf32", bufs=F32_BUFS) as hf32_pool,
            tc.tile_pool(name="g16", bufs=G16_BUFS) as g16_pool,
            tc.tile_pool(name="decp", bufs=2) as dec_pool,
            tc.tile_pool(name="stats", bufs=2) as stats_pool,
            tc.tile_pool(name="small", bufs=2) as small_pool,
            tc.tile_pool(name="singles", bufs=1) as singles,
            tc.tile_pool(name="psum_ctx", bufs=2, space="PSUM") as psum_ctx,
            tc.tile_pool(name="psum_l", bufs=3, space="PSUM") as psum_l,
        ):
            ones_col = singles.tile([P, 1], F32)
            nc.vector.memset(ones_col, 1.0)
            ones_row = singles.tile([1, P], F32)
            nc.vector.memset(ones_row, 1.0)
            # -I[128,128]: transpose-and-negate row maxima on the PE.
            negI = singles.tile([P, P], F32)
            nc.gpsimd.memset(negI, 0.0)
            nc.gpsimd.affine_select(
                out=negI,
                in_=negI,
                compare_op=mybir.AluOpType.not_equal,
                fill=-1.0,
                base=0,
                pattern=[[-1, P]],
                channel_multiplier=1,
            )

            def load_dec(b):
                """Prefetchable per-batch decoder broadcast + 1/dec."""
                dec_rep = dec_pool.tile([P, D], F32, tag="dec")
                nc.scalar.dma_start(
                    out=dec_rep, in_=dec[b : b + 1, :].to_broadcast([P, D])
                )
                # 1/dec for the final division (G carries a factor of dec)
                recip_dec = small_pool.tile([1, D], F32, tag="recip_dec")
                nc.vector.reciprocal(recip_dec, dec_rep[0:1, :])
                return dec_rep, recip_dec

            def one_batch(b, dec_cur, prefetch, pending):
                """Emit one batch using preloaded dec; `prefetch` emits the
                next batch's dec load mid-stream; `pending` holds the
                previous batch's delayed finalization."""
                dec_rep, recip_dec = dec_cur
                score_buf = stats_pool.tile([P, T], F32, tag="score")
                g_tiles = []
                dec_next = None
                for t in range(T):
                    h = hf32_pool.tile([P, D], F32)
                    nc.sync.dma_start(out=h, in_=enc[b, t * P : (t + 1) * P, :])
                    if t == T // 2 and prefetch is not None:
                        dec_next = prefetch()
                    if mode == "dma":
                        continue
                    g = g16_pool.tile([P, D], FP16)
                    g_tiles.append(g)
                    # G16 = H * dec (fp16), scores fused via accum_out
                    nc.vector.scalar_tensor_tensor(
                        out=g,
                        in0=h,
                        scalar=1.0,
                        in1=dec_rep,
                        op0=mybir.AluOpType.bypass,
                        op1=mybir.AluOpType.mult,
                        accum_out=score_buf[:, t : t + 1],
                    )

                # Previous batch's finalization: everything it depends on
                # finished long ago, so it never stalls DVE.
                if pending is not None:
                    pending()

                if mode != "full":
                    zz = small_pool.tile([1, D], F32, tag="zz")
                    nc.vector.memset(zz, 0.0)
                    nc.scalar.dma_start(out=out[b : b + 1, :], in_=zz)
                    return dec_next, None

                # --- batch softmax stats (once per batch) ---
                row_max = small_pool.tile([P, 1], F32, tag="rowmax")
                nc.vector.reduce_max(
                    out=row_max, in_=score_buf, axis=mybir.AxisListType.X
                )
                nrm_t = psum_l.tile([1, P], F32, tag="lp")
                nc.tensor.matmul(nrm_t, lhsT=row_max, rhs=negI, start=True, stop=True)
                neg_m = small_pool.tile([1, 1], F32, tag="neg_m")
                nc.vector.tensor_reduce(
                    out=neg_m, in_=nrm_t, axis=mybir.AxisListType.X,
                    op=mybir.AluOpType.min,
                )
                nm_psum = psum_l.tile([P, 1], F32, tag="lp")
                nc.tensor.matmul(
                    nm_psum, lhsT=ones_row, rhs=neg_m, start=True, stop=True
                )
                neg_m_b = small_pool.tile([P, 1], F32, tag="neg_m_b")
                nc.vector.tensor_copy(out=neg_m_b, in_=nm_psum)

                # attn16 = exp(scores - m) in fp16, with fused row-sum
                attn16 = stats_pool.tile([P, T], FP16, tag="attn16")
                r_p = small_pool.tile([P, 1], F32, tag="r_p")
                nc.scalar.activation(
                    out=attn16,
                    in_=score_buf,
                    func=mybir.ActivationFunctionType.Exp,
                    bias=neg_m_b,
                    scale=1.0,
                    accum_out=r_p,
                )

                # ctx' = sum_t attn16[:,t]^T @ G16[t]  (PSUM [1, D])
                ctx_psum = psum_ctx.tile([1, D], F32, tag="ctx")
                for t in range(T):
                    nc.tensor.matmul(
                        ctx_psum,
                        lhsT=attn16[:, t : t + 1],
                        rhs=g_tiles[t],
                        start=(t == 0),
                        stop=(t == T - 1),
                    )
                l_psum = psum_l.tile([1, 1], F32, tag="lsum")
                nc.tensor.matmul(l_psum, lhsT=r_p, rhs=ones_col, start=True, stop=True)

                def finalize():
                    recip_l = small_pool.tile([1, 1], F32, tag="recip_l")
                    nc.vector.reciprocal(recip_l, l_psum)
                    acc = small_pool.tile([1, D], F32, tag="acc")
                    # ctx = ctx' * (1/L) * (1/dec)
                    nc.vector.scalar_tensor_tensor(
                        out=acc,
                        in0=ctx_psum,
                        scalar=recip_l,
                        in1=recip_dec,
                        op0=mybir.AluOpType.mult,
                        op1=mybir.AluOpType.mult,
                    )
                    nc.scalar.dma_start(out=out[b : b + 1, :], in_=acc)

                return dec_next, finalize

            def one_pass():
                seq = [b for _ in range(repeat) for b in range(B)]
                pending = None
                dec_cur = load_dec(seq[0])
                for i, b in enumerate(seq):
                    nxt = seq[i + 1] if i + 1 < len(seq) else None
                    prefetch = (lambda nb=nxt: load_dec(nb)) if nxt is not None else None
                    dec_next, pending = one_batch(b, dec_cur, prefetch, pending)
                    dec_cur = dec_next
                if pending is not None:
                    pending()

            if loop is not None:
                with tc.For_i(0, loop, 1):
                    one_pass()
            else:
                one_pass()

    _legalize_waits(nc)
    return nc


def _shard(decoder_state: np.ndarray, encoder_hiddens: np.ndarray):
    in_maps = []
    for c in range(N_CORES):
        lo, hi = c * B, (c + 1) * B
        in_maps.append(
            {
                "decoder_state": np.ascontiguousarray(decoder_state[lo:hi]),
                "encoder_hiddens": np.ascontiguousarray(encoder_hiddens[lo:hi]),
            }
        )
    return in_maps


def run(decoder_state: np.ndarray, encoder_hiddens: np.ndarray, trace: bool = False):
    decoder_state = np.asarray(decoder_state, dtype=np.float32)
    encoder_hiddens = np.asarray(encoder_hiddens, dtype=np.float32)
    assert decoder_state.shape == (B_TOTAL, D)
    assert encoder_hiddens.shape == (B_TOTAL, S, D)

    nc = build_nc()
    res = run_bass_kernel_spmd(
        nc, _shard(decoder_state, encoder_hiddens), core_ids=list(range(N_CORES)),
        trace=trace,
    )
    out = np.concatenate([r["context"] for r in res.results], axis=0)
    return out, res


def kernel(decoder_state: np.ndarray, encoder_hiddens: np.ndarray) -> np.ndarray:
    out, _ = run(decoder_state, encoder_hiddens, trace=False)
    return out


# revision 10
# speedup vs baseline: 1.0626x; 1.0626x over previous
"""Luong attention pooling kernel v2 for Trainium2 (Bass/Tile), 8 NeuronCores.

Problem (full shapes, fp32):
    decoder_state:   [32, 512]
    encoder_hiddens: [32, 8192, 512]
    scores  = einsum('bd,bsd->bs'); attn = softmax(scores, axis=1)
    context = einsum('bs,bsd->bd')

Sharding: data-parallel over batch; each of the 8 cores handles 4 batches
independently (no collectives).

Per-core pipeline (HBM-read-bound: ~200 us floor for the 64 MiB stream):
  - stream 64 f32 tiles [128s x 512d] per batch on the SP HWDGE queue
  - per tile, ONE fused DVE op: G16[t] = (H[t] * dec_b) cast to fp16, with
    accum_out giving the 128 scores. G16 is kept in a deep fp16 ring; the
    f32 tile is freed immediately (only DVE reads it).
  - per batch (no segments): rowmax -> PE transpose vs -I -> min = -m ->
    PE broadcast -> ACT exp(scores - m) -> attn16 [128,64] fp16 with fused
    row-sum r_p; L via tiny PE matmul; 64 accumulating PE matmuls
    ctx' = sum_t attn16[:,t]^T @ G16[t]  (= context * dec elementwise)
  - final: ctx = ctx' * (1/L) / dec  (one fused DVE op; division undoes the
    dec factor folded into G; dec ~ N(0,1) so the error stays relative)
  - the final scale/store for batch b is emitted AFTER batch b+1's score
    ops so DVE's in-order stream never stalls on the PE context matmuls.

Engine budgets per iteration (4 batches): DMA ~200 us (floor), DVE ~150 us,
PE ~100 us, ACT ~5 us. Target: the DMA floor.
"""

import numpy as np

import bass_rust
import concourse.bass as bass
import concourse.tile as tile
from concourse import mybir
from concourse.bass_utils import run_bass_kernel_spmd

N_CORES = 8
B_TOTAL = 32
S = 8192
D = 512
B = B_TOTAL // N_CORES
P = 128
T = S // P  # 64 s-tiles per batch

F32 = mybir.dt.float32
FP16 = mybir.dt.float16

F32_BUFS = 40  # staging ring of f32 tiles (2 KiB/partition each)
G16_BUFS = 96  # fp16 G ring (1 KiB/partition each) — 1.5 batches deep


def _legalize_waits(nc: bass.Bass, max_inline: int = 1) -> int:
    """The walrus build here accepts at most one sync wait per regular
    instruction; hoist Tile's multi-waits into standalone EventSemaphore
    instructions immediately before the instruction."""
    n = 0
    for f in nc.m.functions:
        for b in f.blocks:
            changed = False
            new = []
            for i in b.instructions:
                si = i.sync_info
                waits = list(si.on_wait) if si is not None else []
                if len(waits) > max_inline:
                    for k, w in enumerate(waits[max_inline:]):
                        es = mybir.InstEventSemaphore(
                            name=f"{i.name}-hw{k}", engine=i.engine, ins=[], outs=[]
                        )
                        es.sync_info = bass_rust.SyncInfo(on_wait=[w], on_update=[])
                        new.append(es)
                        n += 1
                    i.sync_info = bass_rust.SyncInfo(
                        on_wait=waits[:max_inline], on_update=list(si.on_update)
                    )
                    changed = True
                new.append(i)
            if changed:
                b.instructions = new
    return n


def build_nc(
    repeat: int = 1,
    mode: str = "full",
    loop: int | None = None,
    split_dma: int = 2,
) -> bass.Bass:
    """repeat: unrolled iterations; loop: optional For_i hardware loop around
    them (benchmarking only). mode: "full" | "dma" | "dma+dve".
    split_dma: 0 = all H loads on the SP HWDGE queue; N>0 = every Nth tile
    load issued on the ACT HWDGE queue instead."""
    nc = bass.Bass()
    dec = nc.declare_dram_parameter("decoder_state", [B, D], F32, isOutput=False)
    enc = nc.declare_dram_parameter("encoder_hiddens", [B, S, D], F32, isOutput=False)
    out = nc.declare_dram_parameter("context", [B, D], F32, isOutput=True)

    with tile.TileContext(nc) as tc:
        with (
            tc.tile_pool(name="hf32", bufs=F32_BUFS) as hf32_pool,
            tc.tile_pool(name="g16", bufs=G16_BUFS) as g16_pool,
            tc.tile_pool(name="decp", bufs=2) as dec_pool,
            tc.tile_pool(name="stats", bufs=2) as stats_pool,
            tc.tile_pool(name="small", bufs=2) as small_pool,
            tc.tile_pool(name="singles", bufs=1) as singles,
            tc.tile_pool(name="psum_ctx", bufs=2, space="PSUM") as psum_ctx,
            tc.tile_pool(name="psum_l", bufs=3, space="PSUM") as psum_l,
        ):
            ones_col = singles.tile([P, 1], F32)
            nc.vector.memset(ones_col, 1.0)
            ones_row = singles.tile([1, P], F32)
            nc.vector.memset(ones_row, 1.0)
            # -I[128,128]: transpose-and-negate row maxima on the PE.
            negI = singles.tile([P, P], F32)
            nc.gpsimd.memset(negI, 0.0)
            nc.gpsimd.affine_select(
                out=negI,
                in_=negI,
                compare_op=mybir.AluOpType.not_equal,
                fill=-1.0,
                base=0,
                pattern=[[-1, P]],
                channel_multiplier=1,
            )

            def load_dec(b):
                """Prefetchable per-batch decoder broadcast + 1/dec."""
                dec_rep = dec_pool.tile([P, D], F32, tag="dec")
                nc.scalar.dma_start(
                    out=dec_rep, in_=dec[b : b + 1, :].to_broadcast([P, D])
                )
                # 1/dec for the final division (G carries a factor of dec)
                recip_dec = small_pool.tile([1, D], F32, tag="recip_dec")
                nc.vector.reciprocal(recip_dec, dec_rep[0:1, :])
                return dec_rep, recip_dec

            def one_batch(b, dec_cur, prefetch, pending):
                """Emit one batch using preloaded dec; `prefetch` emits the
                next batch's dec load mid-stream; `pending` holds the
                previous batch's delayed finalization."""
                dec_rep, recip_dec = dec_cur
                score_buf = stats_pool.tile([P, T], F32, tag="score")
                g_tiles = []
                dec_next = None
                for t in range(T):
                    h = hf32_pool.tile([P, D], F32)
                    # t == T-1 stays on SP: the ACT queue also carries the
                    # 256 KiB dec broadcast per batch, so SP takes one extra
                    # H tile to balance the two HWDGE queues.
                    eng = (
                        nc.scalar
                        if split_dma and t % split_dma == split_dma - 1 and t != T - 1
                        else nc.sync
                    )
                    eng.dma_start(out=h, in_=enc[b, t * P : (t + 1) * P, :])
                    if t == T // 2 and prefetch is not None:
                        dec_next = prefetch()
                    if mode == "dma":
                        continue
                    g = g16_pool.tile([P, D], FP16)
                    g_tiles.append(g)
                    # G16 = H * dec (fp16), scores fused via accum_out
                    nc.vector.scalar_tensor_tensor(
                        out=g,
                        in0=h,
                        scalar=1.0,
                        in1=dec_rep,
                        op0=mybir.AluOpType.bypass,
                        op1=mybir.AluOpType.mult,
                        accum_out=score_buf[:, t : t + 1],
                    )

                # Previous batch's finalization: everything it depends on
                # finished long ago, so it never stalls DVE.
                if pending is not None:
                    pending()

                if mode != "full":
                    zz = small_pool.tile([1, D], F32, tag="zz")
                    nc.vector.memset(zz, 0.0)
                    nc.scalar.dma_start(out=out[b : b + 1, :], in_=zz)
                    return dec_next, None

                # --- batch softmax stats (once per batch) ---
                row_max = small_pool.tile([P, 1], F32, tag="rowmax")
                nc.vector.reduce_max(
                    out=row_max, in_=score_buf, axis=mybir.AxisListType.X
                )
                nrm_t = psum_l.tile([1, P], F32, tag="lp")
                nc.tensor.matmul(nrm_t, lhsT=row_max, rhs=negI, start=True, stop=True)
                neg_m = small_pool.tile([1, 1], F32, tag="neg_m")
                nc.vector.tensor_reduce(
                    out=neg_m, in_=nrm_t, axis=mybir.AxisListType.X,
                    op=mybir.AluOpType.min,
                )
                nm_psum = psum_l.tile([P, 1], F32, tag="lp")
                nc.tensor.matmul(
                    nm_psum, lhsT=ones_row, rhs=neg_m, start=True, stop=True
                )
                neg_m_b = small_pool.tile([P, 1], F32, tag="neg_m_b")
                nc.vector.tensor_copy(out=neg_m_b, in_=nm_psum)

                # attn16 = exp(scores - m) in fp16, with fused row-sum
                attn16 = stats_pool.tile([P, T], FP16, tag="attn16")
                r_p = small_pool.tile([P, 1], F32, tag="r_p")
                nc.scalar.activation(
                    out=attn16,
                    in_=score_buf,
                    func=mybir.ActivationFunctionType.Exp,
                    bias=neg_m_b,
                    scale=1.0,
                    accum_out=r_p,
                )

                # ctx' = sum_t attn16[:,t]^T @ G16[t]  (PSUM [1, D])
                ctx_psum = psum_ctx.tile([1, D], F32, tag="ctx")
                for t in range(T):
                    nc.tensor.matmul(
                        ctx_psum,
                        lhsT=attn16[:, t : t + 1],
                        rhs=g_tiles[t],
                        start=(t == 0),
                        stop=(t == T - 1),
                    )
                l_psum = psum_l.tile([1, 1], F32, tag="lsum")
                nc.tensor.matmul(l_psum, lhsT=r_p, rhs=ones_col, start=True, stop=True)

                def finalize():
                    recip_l = small_pool.tile([1, 1], F32, tag="recip_l")
                    nc.vector.reciprocal(recip_l, l_psum)
                    acc = small_pool.tile([1, D], F32, tag="acc")
                    # ctx = ctx' * (1/L) * (1/dec)
                    nc.vector.scalar_tensor_tensor(
                        out=acc,
                        in0=ctx_psum,
                        scalar=recip_l,
                        in1=recip_dec,
                        op0=mybir.AluOpType.mult,
                        op1=mybir.AluOpType.mult,
                    )
                    nc.scalar.dma_start(out=out[b : b + 1, :], in_=acc)

                return dec_next, finalize

            def one_pass():
                seq = [b for _ in range(repeat) for b in range(B)]
                pending = None
                dec_cur = load_dec(seq[0])
                for i, b in enumerate(seq):
                    nxt = seq[i + 1] if i + 1 < len(seq) else None
                    prefetch = (lambda nb=nxt: load_dec(nb)) if nxt is not None else None
                    dec_next, pending = one_batch(b, dec_cur, prefetch, pending)
                    dec_cur = dec_next
                if pending is not None:
                    pending()

            if loop is not None:
                with tc.For_i(0, loop, 1):
                    one_pass()
            else:
                one_pass()

    _legalize_waits(nc)
    return nc


def _shard(decoder_state: np.ndarray, encoder_hiddens: np.ndarray):
    in_maps = []
    for c in range(N_CORES):
        lo, hi = c * B, (c + 1) * B
        in_maps.append(
            {
                "decoder_state": np.ascontiguousarray(decoder_state[lo:hi]),
                "encoder_hiddens": np.ascontiguousarray(encoder_hiddens[lo:hi]),
            }
        )
    return in_maps


def run(decoder_state: np.ndarray, encoder_hiddens: np.ndarray, trace: bool = False):
    decoder_state = np.asarray(decoder_state, dtype=np.float32)
    encoder_hiddens = np.asarray(encoder_hiddens, dtype=np.float32)
    assert decoder_state.shape == (B_TOTAL, D)
    assert encoder_hiddens.shape == (B_TOTAL, S, D)

    nc = build_nc()
    res = run_bass_kernel_spmd(
        nc, _shard(decoder_state, encoder_hiddens), core_ids=list(range(N_CORES)),
        trace=trace,
    )
    out = np.concatenate([r["context"] for r in res.results], axis=0)
    return out, res


def kernel(decoder_state: np.ndarray, encoder_hiddens: np.ndarray) -> np.ndarray:
    out, _ = run(decoder_state, encoder_hiddens, trace=False)
    return out


# revision 13
# speedup vs baseline: 1.0681x; 1.0052x over previous
"""Luong attention pooling kernel v2 for Trainium2 (Bass/Tile), 8 NeuronCores.

Problem (full shapes, fp32):
    decoder_state:   [32, 512]
    encoder_hiddens: [32, 8192, 512]
    scores  = einsum('bd,bsd->bs'); attn = softmax(scores, axis=1)
    context = einsum('bs,bsd->bd')

Sharding: data-parallel over batch; each of the 8 cores handles 4 batches
independently (no collectives).

Per-core pipeline (HBM-read-bound: ~200 us floor for the 64 MiB stream):
  - stream 64 f32 tiles [128s x 512d] per batch on the SP HWDGE queue
  - per tile, ONE fused DVE op: G16[t] = (H[t] * dec_b) cast to fp16, with
    accum_out giving the 128 scores. G16 is kept in a deep fp16 ring; the
    f32 tile is freed immediately (only DVE reads it).
  - per batch (no segments): rowmax -> PE transpose vs -I -> min = -m ->
    PE broadcast -> ACT exp(scores - m) -> attn16 [128,64] fp16 with fused
    row-sum r_p; L via tiny PE matmul; 64 accumulating PE matmuls
    ctx' = sum_t attn16[:,t]^T @ G16[t]  (= context * dec elementwise)
  - final: ctx = ctx' * (1/L) / dec  (one fused DVE op; division undoes the
    dec factor folded into G; dec ~ N(0,1) so the error stays relative)
  - the final scale/store for batch b is emitted AFTER batch b+1's score
    ops so DVE's in-order stream never stalls on the PE context matmuls.

Engine budgets per iteration (4 batches): DMA ~200 us (floor), DVE ~150 us,
PE ~100 us, ACT ~5 us. Target: the DMA floor.
"""

import numpy as np

import bass_rust
import concourse.bass as bass
import concourse.tile as tile
from concourse import mybir
from concourse.bass_utils import run_bass_kernel_spmd

N_CORES = 8
B_TOTAL = 32
S = 8192
D = 512
B = B_TOTAL // N_CORES
P = 128
T = S // P  # 64 s-tiles per batch

F32 = mybir.dt.float32
FP16 = mybir.dt.float16

F32_BUFS = 40  # staging ring of f32 tiles (2 KiB/partition each)
G16_BUFS = 96  # fp16 G ring (1 KiB/partition each) — 1.5 batches deep


def _legalize_waits(nc: bass.Bass, max_inline: int = 1) -> int:
    """The walrus build here accepts at most one sync wait per regular
    instruction; hoist Tile's multi-waits into standalone EventSemaphore
    instructions immediately before the instruction."""
    n = 0
    for f in nc.m.functions:
        for b in f.blocks:
            changed = False
            new = []
            for i in b.instructions:
                si = i.sync_info
                waits = list(si.on_wait) if si is not None else []
                if len(waits) > max_inline:
                    for k, w in enumerate(waits[max_inline:]):
                        es = mybir.InstEventSemaphore(
                            name=f"{i.name}-hw{k}", engine=i.engine, ins=[], outs=[]
                        )
                        es.sync_info = bass_rust.SyncInfo(on_wait=[w], on_update=[])
                        new.append(es)
                        n += 1
                    i.sync_info = bass_rust.SyncInfo(
                        on_wait=waits[:max_inline], on_update=list(si.on_update)
                    )
                    changed = True
                new.append(i)
            if changed:
                b.instructions = new
    return n


def build_nc(
    repeat: int = 1,
    mode: str = "full",
    loop: int | None = None,
    split_dma: int = 2,
    loop_kwargs: dict | None = None,
) -> bass.Bass:
    """repeat: unrolled iterations; loop: optional For_i hardware loop around
    them (benchmarking only). mode: "full" | "dma" | "dma+dve".
    split_dma: 0 = all H loads on the SP HWDGE queue; N>0 = every Nth tile
    load issued on the ACT HWDGE queue instead."""
    nc = bass.Bass()
    dec = nc.declare_dram_parameter("decoder_state", [B, D], F32, isOutput=False)
    enc = nc.declare_dram_parameter("encoder_hiddens", [B, S, D], F32, isOutput=False)
    out = nc.declare_dram_parameter("context", [B, D], F32, isOutput=True)

    with tile.TileContext(nc) as tc:
        with (
            tc.tile_pool(name="hf32", bufs=F32_BUFS) as hf32_pool,
            tc.tile_pool(name="g16", bufs=G16_BUFS) as g16_pool,
            tc.tile_pool(name="decp", bufs=2) as dec_pool,
            tc.tile_pool(name="stats", bufs=2) as stats_pool,
            tc.tile_pool(name="small", bufs=2) as small_pool,
            tc.tile_pool(name="singles", bufs=1) as singles,
            tc.tile_pool(name="psum_ctx", bufs=2, space="PSUM") as psum_ctx,
            tc.tile_pool(name="psum_l", bufs=3, space="PSUM") as psum_l,
        ):
            ones_col = singles.tile([P, 1], F32)
            nc.vector.memset(ones_col, 1.0)
            ones_row = singles.tile([1, P], F32)
            nc.vector.memset(ones_row, 1.0)
            # -I[128,128]: transpose-and-negate row maxima on the PE.
            negI = singles.tile([P, P], F32)
            nc.gpsimd.memset(negI, 0.0)
            nc.gpsimd.affine_select(
                out=negI,
                in_=negI,
                compare_op=mybir.AluOpType.not_equal,
                fill=-1.0,
                base=0,
                pattern=[[-1, P]],
                channel_multiplier=1,
            )

            def load_dec(b):
                """Prefetchable per-batch decoder broadcast + 1/dec."""
                dec_rep = dec_pool.tile([P, D], F32, tag="dec")
                nc.scalar.dma_start(
                    out=dec_rep, in_=dec[b : b + 1, :].to_broadcast([P, D])
                )
                # 1/dec for the final division (G carries a factor of dec)
                recip_dec = small_pool.tile([1, D], F32, tag="recip_dec")
                nc.vector.reciprocal(recip_dec, dec_rep[0:1, :])
                return dec_rep, recip_dec

            def one_batch(b, dec_cur, prefetch, pending):
                """Emit one batch using preloaded dec; `prefetch` emits the
                next batch's dec load mid-stream; `pending` holds the
                previous batch's delayed finalization."""
                dec_rep, recip_dec = dec_cur
                score_buf = stats_pool.tile([P, T], F32, tag="score")
                g_tiles = []
                dec_next = None
                for t in range(T):
                    h = hf32_pool.tile([P, D], F32)
                    # Alternate groups of `split_dma` consecutive tiles
                    # between the two HWDGE queues; consecutive tiles on one
                    # queue keep its HBM reads contiguous. t == T-1 stays on
                    # SP: the ACT queue also carries the 256 KiB dec
                    # broadcast per batch, so SP takes one extra H tile.
                    on_act = (
                        split_dma
                        and (t // split_dma) % 2 == 1
                        and t != T - 1
                    )
                    eng = nc.scalar if on_act else nc.sync
                    eng.dma_start(out=h, in_=enc[b, t * P : (t + 1) * P, :])
                    if t == T // 2 and prefetch is not None:
                        dec_next = prefetch()
                    if mode == "dma":
                        continue
                    g = g16_pool.tile([P, D], FP16)
                    g_tiles.append(g)
                    # G16 = H * dec (fp16), scores fused via accum_out
                    nc.vector.scalar_tensor_tensor(
                        out=g,
                        in0=h,
                        scalar=1.0,
                        in1=dec_rep,
                        op0=mybir.AluOpType.bypass,
                        op1=mybir.AluOpType.mult,
                        accum_out=score_buf[:, t : t + 1],
                    )

                # Previous batch's finalization: everything it depends on
                # finished long ago, so it never stalls DVE.
                if pending is not None:
                    pending()

                if mode != "full":
                    zz = small_pool.tile([1, D], F32, tag="zz")
                    nc.vector.memset(zz, 0.0)
                    nc.scalar.dma_start(out=out[b : b + 1, :], in_=zz)
                    return dec_next, None

                # --- batch softmax stats (once per batch) ---
                row_max = small_pool.tile([P, 1], F32, tag="rowmax")
                nc.vector.reduce_max(
                    out=row_max, in_=score_buf, axis=mybir.AxisListType.X
                )
                nrm_t = psum_l.tile([1, P], F32, tag="lp")
                nc.tensor.matmul(nrm_t, lhsT=row_max, rhs=negI, start=True, stop=True)
                neg_m = small_pool.tile([1, 1], F32, tag="neg_m")
                nc.vector.tensor_reduce(
                    out=neg_m, in_=nrm_t, axis=mybir.AxisListType.X,
                    op=mybir.AluOpType.min,
                )
                nm_psum = psum_l.tile([P, 1], F32, tag="lp")
                nc.tensor.matmul(
                    nm_psum, lhsT=ones_row, rhs=neg_m, start=True, stop=True
                )
                neg_m_b = small_pool.tile([P, 1], F32, tag="neg_m_b")
                nc.vector.tensor_copy(out=neg_m_b, in_=nm_psum)

                # attn16 = exp(scores - m) in fp16, with fused row-sum
                attn16 = stats_pool.tile([P, T], FP16, tag="attn16")
                r_p = small_pool.tile([P, 1], F32, tag="r_p")
                nc.scalar.activation(
                    out=attn16,
                    in_=score_buf,
                    func=mybir.ActivationFunctionType.Exp,
                    bias=neg_m_b,
                    scale=1.0,
                    accum_out=r_p,
                )

                # ctx' = sum_t attn16[:,t]^T @ G16[t]  (PSUM [1, D])
                ctx_psum = psum_ctx.tile([1, D], F32, tag="ctx")
                for t in range(T):
                    nc.tensor.matmul(
                        ctx_psum,
                        lhsT=attn16[:, t : t + 1],
                        rhs=g_tiles[t],
                        start=(t == 0),
                        stop=(t == T - 1),
                    )
                l_psum = psum_l.tile([1, 1], F32, tag="lsum")
                nc.tensor.matmul(l_psum, lhsT=r_p, rhs=ones_col, start=True, stop=True)

                def finalize():
                    recip_l = small_pool.tile([1, 1], F32, tag="recip_l")
                    nc.vector.reciprocal(recip_l, l_psum)
                    acc = small_pool.tile([1, D], F32, tag="acc")
                    # ctx = ctx' * (1/L) * (1/dec)
                    nc.vector.scalar_tensor_tensor(
                        out=acc,
                        in0=ctx_psum,
                        scalar=recip_l,
                        in1=recip_dec,
                        op0=mybir.AluOpType.mult,
                        op1=mybir.AluOpType.mult,
                    )
                    nc.scalar.dma_start(out=out[b : b + 1, :], in_=acc)

                return dec_next, finalize

            def one_pass():
                seq = [b for _ in range(repeat) for b in range(B)]
                pending = None
                dec_cur = load_dec(seq[0])
                for i, b in enumerate(seq):
                    nxt = seq[i + 1] if i + 1 < len(seq) else None
                    prefetch = (lambda nb=nxt: load_dec(nb)) if nxt is not None else None
                    dec_next, pending = one_batch(b, dec_cur, prefetch, pending)
                    dec_cur = dec_next
                if pending is not None:
                    pending()

            if loop is not None:
                with tc.For_i(0, loop, 1, **(loop_kwargs or {})):
                    one_pass()
            else:
                one_pass()

    _legalize_waits(nc)
    return nc


def _shard(decoder_state: np.ndarray, encoder_hiddens: np.ndarray):
    in_maps = []
    for c in range(N_CORES):
        lo, hi = c * B, (c + 1) * B
        in_maps.append(
            {
                "decoder_state": np.ascontiguousarray(decoder_state[lo:hi]),
                "encoder_hiddens": np.ascontiguousarray(encoder_hiddens[lo:hi]),
            }
        )
    return in_maps


def run(decoder_state: np.ndarray, encoder_hiddens: np.ndarray, trace: bool = False):
    decoder_state = np.asarray(decoder_state, dtype=np.float32)
    encoder_hiddens = np.asarray(encoder_hiddens, dtype=np.float32)
    assert decoder_state.shape == (B_TOTAL, D)
    assert encoder_hiddens.shape == (B_TOTAL, S, D)

    nc = build_nc()
    res = run_bass_kernel_spmd(
        nc, _shard(decoder_state, encoder_hiddens), core_ids=list(range(N_CORES)),
        trace=trace,
    )
    out = np.concatenate([r["context"] for r in res.results], axis=0)
    return out, res


def kernel(decoder_state: np.ndarray, encoder_hiddens: np.ndarray) -> np.ndarray:
    out, _ = run(decoder_state, encoder_hiddens, trace=False)
    return out
